# revision 7
# baseline (speedup 1.0000x reference)
"""Trainium2 Bass kernel for KroneckerLinear: y = x @ kron(U, V).

Math: with x[t] reshaped to X_t [i1=128, i2=128] (i2 contiguous) and
y[t] reshaped to Y_t [j1=128, j2=128] (j2 contiguous):

    Y_t = U^T @ X_t @ V

fp16t mode (default):
  Host pre-transposes x to [i2, t, i1] fp16 and post-transposes
  y from [j1, t, j2] fp16, so every DMA moves 8 KB contiguous per
  partition (no 512 B strided chunks), at half the bytes of fp32.

    MM1: lhsT = X_t^T [i2, i1] (per-token stationary),
         rhs  = V [i2, j2]     -> Z_t = X_t V   [i1, j2] (PSUM)
    Z copy: PSUM -> SBUF fp16 (DVE)
    MM2: lhsT = U [i1, j1] (constant stationary),
         rhs  = Z block [i1, quad*j2] (N=512)   -> Y block [j1, quad*j2]
    Y copy: PSUM -> SBUF fp16 (ACT)

Sharding: data-parallel over the token dim, 256 tokens per core x 8 cores.
"""

import sys

if "/opt/trn_rl_repo" not in sys.path:
    sys.path.insert(0, "/opt/trn_rl_repo")

import numpy as np

import concourse.bacc as bacc
import concourse.bass as bass
import concourse.mybir as mybir
from concourse import tile
from concourse.bass_utils import run_bass_kernel_spmd

F32 = mybir.dt.float32
F16 = mybir.dt.float16

N_CORES = 8
TOKENS = 2048
D = 16384  # 128 * 128
T_CORE = TOKENS // N_CORES  # 256


def build_nc_fp16t(n_tokens=T_CORE, group=64, quad=4, store_eng="scalar",
                   oct_=8, sgroup=32, delay=1):
    """fp16, host-transposed IO, constant-stationary second matmul.

    Processes "octs" of 8 tokens: 8 MM1s into one 2-bank PSUM tile, one
    FD=1024 DVE cast, two N=512 MM2s into a second 2-bank PSUM tile, one
    FD=1024 ACT copy. MM2+Y-copy of oct o-delay issue between the MM1s
    of oct o and its cast so the tensor queue doesn't stall on cast
    latency. Loads are `group`-token (2 MiB) transfers; stores are
    `sgroup`-token (1 MiB) transfers to smooth HBM write traffic.
    """
    OCT = oct_
    assert n_tokens % group == 0 and group % sgroup == 0 and sgroup % OCT == 0
    octs_per_group = group // OCT
    octs_per_sgroup = sgroup // OCT
    n_octs = n_tokens // OCT

    nc = bacc.Bacc("TRN2", target_bir_lowering=False, debug=False)
    x = nc.dram_tensor("x", [128, n_tokens, 128], F16, kind="ExternalInput")
    u = nc.dram_tensor("u", [128, 128], F16, kind="ExternalInput")
    v = nc.dram_tensor("v", [128, 128], F16, kind="ExternalInput")
    y = nc.dram_tensor("y", [128, n_tokens, 128], F16, kind="ExternalOutput")

    with tile.TileContext(nc) as tc:
        with (
            tc.tile_pool(name="const", bufs=1) as cpool,
            tc.tile_pool(name="xin", bufs=3) as xpool,
            tc.tile_pool(name="yout", bufs=3) as ypool,
            tc.tile_pool(name="zmid", bufs=delay + 3) as zpool,
            tc.tile_pool(name="psA", bufs=2, space="PSUM") as papool,
            tc.tile_pool(name="psB", bufs=2, space="PSUM") as pbpool,
        ):
            u_sb = cpool.tile([128, 128], F16)
            v_sb = cpool.tile([128, 128], F16)
            nc.sync.dma_start(u_sb[:], u[:])
            nc.sync.dma_start(v_sb[:], v[:])
            st = getattr(nc, store_eng)

            yts = {}
            zsbs = {}
            xts = {}

            def mm2_and_evacuate(o):
                """Second matmuls + Y evacuation for oct o (deferred)."""
                s, so = divmod(o, octs_per_sgroup)
                zsb = zsbs.pop(o)
                yb = pbpool.tile([128, OCT, 128], F32)
                for h in range(OCT // 4):
                    nc.tensor.matmul(
                        yb[:, h * 4 : (h + 1) * 4, :],
                        lhsT=u_sb[:],
                        rhs=zsb[:, h * 4 : (h + 1) * 4, :],
                        start=True,
                        stop=True,
                    )
                nc.scalar.copy(yts[s][:, so * OCT : (so + 1) * OCT, :], yb[:])
                if so == octs_per_sgroup - 1:
                    st.dma_start(
                        y[:, s * sgroup : (s + 1) * sgroup, :], yts.pop(s)[:]
                    )

            for o in range(n_octs):
                g, og = divmod(o, octs_per_group)
                s = o // octs_per_sgroup
                if og == 0:
                    xt = xpool.tile([128, group, 128], F16, name="xt")
                    nc.sync.dma_start(xt[:], x[:, g * group : (g + 1) * group, :])
                    xts[g] = xt
                if o % octs_per_sgroup == 0:
                    yts[s] = ypool.tile([128, sgroup, 128], F16, name="yt")
                xt = xts[g]
                za = papool.tile([128, OCT, 128], F32)
                for j in range(OCT):
                    nc.tensor.matmul(
                        za[:, j, :],
                        lhsT=xt[:, og * OCT + j, :],
                        rhs=v_sb[:],
                        start=True,
                        stop=True,
                    )
                if o >= delay:
                    mm2_and_evacuate(o - delay)
                zsb = zpool.tile([128, OCT, 128], F16)
                nc.vector.tensor_copy(zsb[:], za[:])
                zsbs[o] = zsb
            for o in range(n_octs - delay, n_octs):
                mm2_and_evacuate(o)
    nc.compile()
    return nc


# ---------------------------------------------------------------------------
# fp32r baseline (previous best) kept for A/B comparison.

F32R = mybir.dt.float32r


def build_nc(n_tokens=T_CORE, mode="fp32rh", group=32, quad=4, lgran=0, sgran=0):
    r = mode in ("fp32r", "fp32rh")
    hostround = mode == "fp32rh"
    rwide = 256 if r else 128
    mmdt = F32R if r else F32
    dramdt = F32R if hostround else F32
    lgran = lgran or group
    sgran = sgran or group
    assert group % lgran == 0 and group % sgran == 0

    nc = bacc.Bacc("TRN2", target_bir_lowering=False, debug=False)
    x = nc.dram_tensor("x", [n_tokens, D], dramdt, kind="ExternalInput")
    u = nc.dram_tensor("u", [128, rwide], dramdt, kind="ExternalInput")
    v = nc.dram_tensor("v", [128, rwide], dramdt, kind="ExternalInput")
    y = nc.dram_tensor("y", [n_tokens, D], F32, kind="ExternalOutput")

    with tile.TileContext(nc) as tc:
        with (
            tc.tile_pool(name="const", bufs=1) as cpool,
            tc.tile_pool(name="xin", bufs=2) as xpool,
            tc.tile_pool(name="yout", bufs=2) as ypool,
            tc.tile_pool(name="pmid", bufs=4) as ppool,
            tc.tile_pool(name="ps", bufs=2, space="PSUM") as pspool,
        ):
            u_sb = cpool.tile([128, rwide], mmdt)
            v_sb = cpool.tile([128, rwide], mmdt)
            ld_const = (
                nc.gpsimd.dma_start if (r and not hostround) else nc.sync.dma_start
            )
            ld_const(u_sb[:], u[:])
            ld_const(v_sb[:], v[:])

            for g in range(n_tokens // group):
                xt = xpool.tile([128, group, 128], mmdt)
                ld_x = (
                    nc.gpsimd.dma_start if (r and not hostround) else nc.sync.dma_start
                )
                for c in range(group // lgran):
                    t0 = g * group + c * lgran
                    ld_x(
                        xt[:, c * lgran : (c + 1) * lgran, :],
                        x[t0 : t0 + lgran].rearrange("t (i1 i2) -> i1 t i2", i1=128),
                    )
                yt = ypool.tile([128, group, 128], F32)
                for q in range(group // quad):
                    pa = pspool.tile([128, quad, rwide], F32)
                    for j in range(quad):
                        nc.tensor.matmul(
                            pa[:, j, :],
                            lhsT=xt[:, q * quad + j, :],
                            rhs=u_sb[:],
                            start=True,
                            stop=True,
                        )
                    psb = ppool.tile([128, quad, 128], mmdt)
                    nc.vector.tensor_copy(psb[:], pa[:, :, 0:128])
                    pb = pspool.tile([128, quad, rwide], F32)
                    for j in range(quad):
                        nc.tensor.matmul(
                            pb[:, j, :],
                            lhsT=psb[:, j, :],
                            rhs=v_sb[:],
                            start=True,
                            stop=True,
                        )
                    nc.vector.tensor_copy(
                        yt[:, q * quad : (q + 1) * quad, :], pb[:, :, 0:128]
                    )
                for c in range(group // sgran):
                    t0 = g * group + c * sgran
                    nc.scalar.dma_start(
                        y[t0 : t0 + sgran].rearrange("t (j1 j2) -> j1 t j2", j1=128),
                        yt[:, c * sgran : (c + 1) * sgran, :],
                    )
    nc.compile()
    return nc


_NC_CACHE = {}


def _get_nc(n_tokens, mode, group, quad, lgran, sgran, store_eng):
    key = (n_tokens, mode, group, quad, lgran, sgran, store_eng)
    if key not in _NC_CACHE:
        if mode == "fp16t":
            _NC_CACHE[key] = build_nc_fp16t(n_tokens, group, quad, store_eng)
        else:
            _NC_CACHE[key] = build_nc(n_tokens, mode, group, quad, lgran, sgran)
    return _NC_CACHE[key]


def round_fp32r(a):
    u = np.ascontiguousarray(a, dtype=np.float32).view(np.uint32)
    r = ((u + np.uint32(0x800)) & np.uint32(0xFFFFF000)).view(np.float32)
    return np.where(np.isfinite(a), r, a).astype(np.float32)


def _prep_inputs(x, U, V, mode):
    x = np.ascontiguousarray(np.asarray(x), dtype=np.float32)
    U = np.ascontiguousarray(np.asarray(U), dtype=np.float32)
    V = np.ascontiguousarray(np.asarray(V), dtype=np.float32)
    if mode in ("fp32r", "fp32rh"):
        U = np.concatenate([U, U], axis=1)
        V = np.concatenate([V, V], axis=1)
    if mode == "fp32rh":
        x = round_fp32r(x)
        U = round_fp32r(U)
        V = round_fp32r(V)
    return x, U, V


def run(x, U, V, mode="fp16t", group=32, quad=4, lgran=0, sgran=0,
        store_eng="scalar", trace=False, **spmd_kwargs):
    """Shard over 8 cores, run, gather. Returns (y_full, BassKernelResults)."""
    if mode == "fp16t":
        x = np.ascontiguousarray(np.asarray(x), dtype=np.float32)
        T = x.shape[0]
        t_core = T // N_CORES
        nc = _get_nc(t_core, mode, group, quad, 0, 0, store_eng)
        # [t, i1*128+i2] -> [i2, t, i1] fp16
        xt = np.ascontiguousarray(
            x.reshape(T, 128, 128).transpose(2, 0, 1).astype(np.float16)
        )
        Uh = np.ascontiguousarray(np.asarray(U), dtype=np.float16)
        Vh = np.ascontiguousarray(np.asarray(V), dtype=np.float16)
        in_maps = [
            {
                "x": np.ascontiguousarray(xt[:, i * t_core : (i + 1) * t_core, :]),
                "u": Uh,
                "v": Vh,
            }
            for i in range(N_CORES)
        ]
        res = run_bass_kernel_spmd(
            nc, in_maps, list(range(N_CORES)), trace=trace, **spmd_kwargs
        )
        # y core result: [j1, t, j2] -> [t, j1, j2]
        out = np.concatenate(
            [
                res.results[i]["y"].transpose(1, 0, 2).reshape(t_core, D)
                for i in range(N_CORES)
            ],
            axis=0,
        ).astype(np.float32)
        return out, res

    x, U, V = _prep_inputs(x, U, V, mode)
    t_core = x.shape[0] // N_CORES
    nc = _get_nc(t_core, mode, group, quad, lgran, sgran, store_eng)
    in_maps = [
        {"x": x[i * t_core : (i + 1) * t_core], "u": U, "v": V}
        for i in range(N_CORES)
    ]
    res = run_bass_kernel_spmd(
        nc, in_maps, list(range(N_CORES)), trace=trace, **spmd_kwargs
    )
    out = np.concatenate([res.results[i]["y"] for i in range(N_CORES)], axis=0)
    return out, res


def kernel(x, U, V):
    out, _ = run(x, U, V)
    return out


# revision 8
# speedup vs baseline: 1.0011x; 1.0011x over previous
"""Trainium2 Bass kernel for KroneckerLinear: y = x @ kron(U, V).

Math: with x[t] reshaped to X_t [i1=128, i2=128] (i2 contiguous) and
y[t] reshaped to Y_t [j1=128, j2=128] (j2 contiguous):

    Y_t = U^T @ X_t @ V

fp16t mode (default):
  Host pre-transposes x to [i2, t, i1] fp16 and post-transposes
  y from [j1, t, j2] fp16, so every DMA moves 8 KB contiguous per
  partition (no 512 B strided chunks), at half the bytes of fp32.

    MM1: lhsT = X_t^T [i2, i1] (per-token stationary),
         rhs  = V [i2, j2]     -> Z_t = X_t V   [i1, j2] (PSUM)
    Z copy: PSUM -> SBUF fp16 (DVE)
    MM2: lhsT = U [i1, j1] (constant stationary),
         rhs  = Z block [i1, quad*j2] (N=512)   -> Y block [j1, quad*j2]
    Y copy: PSUM -> SBUF fp16 (ACT)

Sharding: data-parallel over the token dim, 256 tokens per core x 8 cores.
"""

import sys

if "/opt/trn_rl_repo" not in sys.path:
    sys.path.insert(0, "/opt/trn_rl_repo")

import numpy as np

import concourse.bacc as bacc
import concourse.bass as bass
import concourse.mybir as mybir
from concourse import tile
from concourse.bass_utils import run_bass_kernel_spmd

F32 = mybir.dt.float32
F16 = mybir.dt.float16

N_CORES = 8
TOKENS = 2048
D = 16384  # 128 * 128
T_CORE = TOKENS // N_CORES  # 256


def build_nc_fp16t(n_tokens=T_CORE, group=64, quad=4, store_eng="scalar",
                   oct_=8, sgroup=32, delay=1):
    """fp16, host-transposed IO, constant-stationary second matmul.

    Processes "octs" of 8 tokens: 8 MM1s into one 2-bank PSUM tile, one
    FD=1024 DVE cast, two N=512 MM2s into a second 2-bank PSUM tile, one
    FD=1024 ACT copy. MM2+Y-copy of oct o-delay issue between the MM1s
    of oct o and its cast so the tensor queue doesn't stall on cast
    latency. Loads are `group`-token (2 MiB) transfers; stores are
    `sgroup`-token (1 MiB) transfers to smooth HBM write traffic.
    """
    OCT = oct_
    assert n_tokens % group == 0 and group % sgroup == 0 and sgroup % OCT == 0
    octs_per_group = group // OCT
    octs_per_sgroup = sgroup // OCT
    n_octs = n_tokens // OCT

    nc = bacc.Bacc("TRN2", target_bir_lowering=False, debug=False)
    x = nc.dram_tensor("x", [128, n_tokens, 128], F16, kind="ExternalInput")
    u = nc.dram_tensor("u", [128, 128], F16, kind="ExternalInput")
    v = nc.dram_tensor("v", [128, 128], F16, kind="ExternalInput")
    y = nc.dram_tensor("y", [128, n_tokens, 128], F16, kind="ExternalOutput")

    with tile.TileContext(nc) as tc:
        with (
            tc.tile_pool(name="const", bufs=1) as cpool,
            tc.tile_pool(name="xin", bufs=3) as xpool,
            tc.tile_pool(name="yout", bufs=6) as ypool,
            tc.tile_pool(name="zmid", bufs=delay + 3) as zpool,
            tc.tile_pool(name="psA", bufs=2, space="PSUM") as papool,
            tc.tile_pool(name="psB", bufs=2, space="PSUM") as pbpool,
        ):
            u_sb = cpool.tile([128, 128], F16)
            v_sb = cpool.tile([128, 128], F16)
            nc.sync.dma_start(u_sb[:], u[:])
            nc.sync.dma_start(v_sb[:], v[:])
            st = getattr(nc, store_eng)

            yts = {}
            zsbs = {}
            xts = {}

            def mm2_and_evacuate(o):
                """Second matmuls + Y evacuation for oct o (deferred)."""
                s, so = divmod(o, octs_per_sgroup)
                zsb = zsbs.pop(o)
                yb = pbpool.tile([128, OCT, 128], F32)
                for h in range(OCT // 4):
                    nc.tensor.matmul(
                        yb[:, h * 4 : (h + 1) * 4, :],
                        lhsT=u_sb[:],
                        rhs=zsb[:, h * 4 : (h + 1) * 4, :],
                        start=True,
                        stop=True,
                    )
                nc.scalar.copy(yts[s][:, so * OCT : (so + 1) * OCT, :], yb[:])
                if so == octs_per_sgroup - 1:
                    st_eng = st if s % 2 == 0 else nc.gpsimd
                    st_eng.dma_start(
                        y[:, s * sgroup : (s + 1) * sgroup, :], yts.pop(s)[:]
                    )

            for o in range(n_octs):
                g, og = divmod(o, octs_per_group)
                s = o // octs_per_sgroup
                if og == 0:
                    xt = xpool.tile([128, group, 128], F16, name="xt")
                    if g == 0:
                        # small first chunk so compute starts ASAP
                        nc.sync.dma_start(xt[:, :OCT, :], x[:, :OCT, :])
                        nc.sync.dma_start(
                            xt[:, OCT:, :], x[:, OCT : group, :]
                        )
                    else:
                        nc.sync.dma_start(
                            xt[:], x[:, g * group : (g + 1) * group, :]
                        )
                    xts[g] = xt
                if o % octs_per_sgroup == 0:
                    yts[s] = ypool.tile([128, sgroup, 128], F16, name="yt")
                xt = xts[g]
                za = papool.tile([128, OCT, 128], F32)
                for j in range(OCT):
                    nc.tensor.matmul(
                        za[:, j, :],
                        lhsT=xt[:, og * OCT + j, :],
                        rhs=v_sb[:],
                        start=True,
                        stop=True,
                    )
                if o >= delay:
                    mm2_and_evacuate(o - delay)
                zsb = zpool.tile([128, OCT, 128], F16)
                nc.vector.tensor_copy(zsb[:], za[:])
                zsbs[o] = zsb
            for o in range(n_octs - delay, n_octs):
                mm2_and_evacuate(o)
    nc.compile()
    return nc


# ---------------------------------------------------------------------------
# fp32r baseline (previous best) kept for A/B comparison.

F32R = mybir.dt.float32r


def build_nc(n_tokens=T_CORE, mode="fp32rh", group=32, quad=4, lgran=0, sgran=0):
    r = mode in ("fp32r", "fp32rh")
    hostround = mode == "fp32rh"
    rwide = 256 if r else 128
    mmdt = F32R if r else F32
    dramdt = F32R if hostround else F32
    lgran = lgran or group
    sgran = sgran or group
    assert group % lgran == 0 and group % sgran == 0

    nc = bacc.Bacc("TRN2", target_bir_lowering=False, debug=False)
    x = nc.dram_tensor("x", [n_tokens, D], dramdt, kind="ExternalInput")
    u = nc.dram_tensor("u", [128, rwide], dramdt, kind="ExternalInput")
    v = nc.dram_tensor("v", [128, rwide], dramdt, kind="ExternalInput")
    y = nc.dram_tensor("y", [n_tokens, D], F32, kind="ExternalOutput")

    with tile.TileContext(nc) as tc:
        with (
            tc.tile_pool(name="const", bufs=1) as cpool,
            tc.tile_pool(name="xin", bufs=2) as xpool,
            tc.tile_pool(name="yout", bufs=2) as ypool,
            tc.tile_pool(name="pmid", bufs=4) as ppool,
            tc.tile_pool(name="ps", bufs=2, space="PSUM") as pspool,
        ):
            u_sb = cpool.tile([128, rwide], mmdt)
            v_sb = cpool.tile([128, rwide], mmdt)
            ld_const = (
                nc.gpsimd.dma_start if (r and not hostround) else nc.sync.dma_start
            )
            ld_const(u_sb[:], u[:])
            ld_const(v_sb[:], v[:])

            for g in range(n_tokens // group):
                xt = xpool.tile([128, group, 128], mmdt)
                ld_x = (
                    nc.gpsimd.dma_start if (r and not hostround) else nc.sync.dma_start
                )
                for c in range(group // lgran):
                    t0 = g * group + c * lgran
                    ld_x(
                        xt[:, c * lgran : (c + 1) * lgran, :],
                        x[t0 : t0 + lgran].rearrange("t (i1 i2) -> i1 t i2", i1=128),
                    )
                yt = ypool.tile([128, group, 128], F32)
                for q in range(group // quad):
                    pa = pspool.tile([128, quad, rwide], F32)
                    for j in range(quad):
                        nc.tensor.matmul(
                            pa[:, j, :],
                            lhsT=xt[:, q * quad + j, :],
                            rhs=u_sb[:],
                            start=True,
                            stop=True,
                        )
                    psb = ppool.tile([128, quad, 128], mmdt)
                    nc.vector.tensor_copy(psb[:], pa[:, :, 0:128])
                    pb = pspool.tile([128, quad, rwide], F32)
                    for j in range(quad):
                        nc.tensor.matmul(
                            pb[:, j, :],
                            lhsT=psb[:, j, :],
                            rhs=v_sb[:],
                            start=True,
                            stop=True,
                        )
                    nc.vector.tensor_copy(
                        yt[:, q * quad : (q + 1) * quad, :], pb[:, :, 0:128]
                    )
                for c in range(group // sgran):
                    t0 = g * group + c * sgran
                    nc.scalar.dma_start(
                        y[t0 : t0 + sgran].rearrange("t (j1 j2) -> j1 t j2", j1=128),
                        yt[:, c * sgran : (c + 1) * sgran, :],
                    )
    nc.compile()
    return nc


_NC_CACHE = {}


def _get_nc(n_tokens, mode, group, quad, lgran, sgran, store_eng):
    key = (n_tokens, mode, group, quad, lgran, sgran, store_eng)
    if key not in _NC_CACHE:
        if mode == "fp16t":
            _NC_CACHE[key] = build_nc_fp16t(n_tokens, group, quad, store_eng)
        else:
            _NC_CACHE[key] = build_nc(n_tokens, mode, group, quad, lgran, sgran)
    return _NC_CACHE[key]


def round_fp32r(a):
    u = np.ascontiguousarray(a, dtype=np.float32).view(np.uint32)
    r = ((u + np.uint32(0x800)) & np.uint32(0xFFFFF000)).view(np.float32)
    return np.where(np.isfinite(a), r, a).astype(np.float32)


def _prep_inputs(x, U, V, mode):
    x = np.ascontiguousarray(np.asarray(x), dtype=np.float32)
    U = np.ascontiguousarray(np.asarray(U), dtype=np.float32)
    V = np.ascontiguousarray(np.asarray(V), dtype=np.float32)
    if mode in ("fp32r", "fp32rh"):
        U = np.concatenate([U, U], axis=1)
        V = np.concatenate([V, V], axis=1)
    if mode == "fp32rh":
        x = round_fp32r(x)
        U = round_fp32r(U)
        V = round_fp32r(V)
    return x, U, V


def run(x, U, V, mode="fp16t", group=32, quad=4, lgran=0, sgran=0,
        store_eng="scalar", trace=False, **spmd_kwargs):
    """Shard over 8 cores, run, gather. Returns (y_full, BassKernelResults)."""
    if mode == "fp16t":
        x = np.ascontiguousarray(np.asarray(x), dtype=np.float32)
        T = x.shape[0]
        t_core = T // N_CORES
        nc = _get_nc(t_core, mode, group, quad, 0, 0, store_eng)
        # [t, i1*128+i2] -> [i2, t, i1] fp16
        xt = np.ascontiguousarray(
            x.reshape(T, 128, 128).transpose(2, 0, 1).astype(np.float16)
        )
        Uh = np.ascontiguousarray(np.asarray(U), dtype=np.float16)
        Vh = np.ascontiguousarray(np.asarray(V), dtype=np.float16)
        in_maps = [
            {
                "x": np.ascontiguousarray(xt[:, i * t_core : (i + 1) * t_core, :]),
                "u": Uh,
                "v": Vh,
            }
            for i in range(N_CORES)
        ]
        res = run_bass_kernel_spmd(
            nc, in_maps, list(range(N_CORES)), trace=trace, **spmd_kwargs
        )
        # y core result: [j1, t, j2] -> [t, j1, j2]
        out = np.concatenate(
            [
                res.results[i]["y"].transpose(1, 0, 2).reshape(t_core, D)
                for i in range(N_CORES)
            ],
            axis=0,
        ).astype(np.float32)
        return out, res

    x, U, V = _prep_inputs(x, U, V, mode)
    t_core = x.shape[0] // N_CORES
    nc = _get_nc(t_core, mode, group, quad, lgran, sgran, store_eng)
    in_maps = [
        {"x": x[i * t_core : (i + 1) * t_core], "u": U, "v": V}
        for i in range(N_CORES)
    ]
    res = run_bass_kernel_spmd(
        nc, in_maps, list(range(N_CORES)), trace=trace, **spmd_kwargs
    )
    out = np.concatenate([res.results[i]["y"] for i in range(N_CORES)], axis=0)
    return out, res


def kernel(x, U, V):
    out, _ = run(x, U, V)
    return out


# revision 9
# speedup vs baseline: 1.0638x; 1.0627x over previous
"""Trainium2 Bass kernel for KroneckerLinear: y = x @ kron(U, V).

Math: with x[t] reshaped to X_t [i1=128, i2=128] (i2 contiguous) and
y[t] reshaped to Y_t [j1=128, j2=128] (j2 contiguous):

    Y_t = U^T @ X_t @ V

fp16t mode (default):
  Host pre-transposes x to [i2, t, i1] fp16 and post-transposes
  y from [j1, t, j2] fp16, so every DMA moves 8 KB contiguous per
  partition (no 512 B strided chunks), at half the bytes of fp32.

    MM1: lhsT = X_t^T [i2, i1] (per-token stationary),
         rhs  = V [i2, j2]     -> Z_t = X_t V   [i1, j2] (PSUM)
    Z copy: PSUM -> SBUF fp16 (DVE)
    MM2: lhsT = U [i1, j1] (constant stationary),
         rhs  = Z block [i1, quad*j2] (N=512)   -> Y block [j1, quad*j2]
    Y copy: PSUM -> SBUF fp16 (ACT)

Sharding: data-parallel over the token dim, 256 tokens per core x 8 cores.
"""

import sys

if "/opt/trn_rl_repo" not in sys.path:
    sys.path.insert(0, "/opt/trn_rl_repo")

import numpy as np

import concourse.bacc as bacc
import concourse.bass as bass
import concourse.mybir as mybir
from concourse import tile
from concourse.bass_utils import run_bass_kernel_spmd

F32 = mybir.dt.float32
F16 = mybir.dt.float16

N_CORES = 8
TOKENS = 2048
D = 16384  # 128 * 128
T_CORE = TOKENS // N_CORES  # 256


def build_nc_fp16t(n_tokens=T_CORE, group=32, quad=4, store_eng="scalar",
                   oct_=8, sgroup=32, delay=2):
    """fp16, host-transposed IO, constant-stationary second matmul.

    Processes "octs" of 8 tokens: 8 MM1s into one 2-bank PSUM tile, one
    FD=1024 DVE cast, two N=512 MM2s into a second 2-bank PSUM tile, one
    FD=1024 ACT copy. MM2+Y-copy of oct o-delay issue between the MM1s
    of oct o and its cast so the tensor queue doesn't stall on cast
    latency. Loads are `group`-token (2 MiB) transfers; stores are
    `sgroup`-token (1 MiB) transfers to smooth HBM write traffic.
    """
    OCT = oct_
    assert n_tokens % group == 0 and group % sgroup == 0 and sgroup % OCT == 0
    octs_per_group = group // OCT
    octs_per_sgroup = sgroup // OCT
    n_octs = n_tokens // OCT

    nc = bacc.Bacc("TRN2", target_bir_lowering=False, debug=False)
    x = nc.dram_tensor("x", [128, n_tokens, 128], F16, kind="ExternalInput")
    u = nc.dram_tensor("u", [128, 128], F16, kind="ExternalInput")
    v = nc.dram_tensor("v", [128, 128], F16, kind="ExternalInput")
    y = nc.dram_tensor("y", [128, n_tokens, 128], F16, kind="ExternalOutput")

    with tile.TileContext(nc) as tc:
        with (
            tc.tile_pool(name="const", bufs=1) as cpool,
            tc.tile_pool(name="xin", bufs=6) as xpool,
            tc.tile_pool(name="yout", bufs=6) as ypool,
            tc.tile_pool(name="zmid", bufs=delay + 3) as zpool,
            tc.tile_pool(name="psA", bufs=2, space="PSUM") as papool,
            tc.tile_pool(name="psB", bufs=2, space="PSUM") as pbpool,
        ):
            u_sb = cpool.tile([128, 128], F16)
            v_sb = cpool.tile([128, 128], F16)
            nc.sync.dma_start(u_sb[:], u[:])
            nc.sync.dma_start(v_sb[:], v[:])
            st = getattr(nc, store_eng)

            yts = {}
            zsbs = {}
            xts = {}

            def mm2_and_evacuate(o):
                """Second matmuls + Y evacuation for oct o (deferred)."""
                s, so = divmod(o, octs_per_sgroup)
                zsb = zsbs.pop(o)
                yb = pbpool.tile([128, OCT, 128], F32)
                for h in range(OCT // 4):
                    nc.tensor.matmul(
                        yb[:, h * 4 : (h + 1) * 4, :],
                        lhsT=u_sb[:],
                        rhs=zsb[:, h * 4 : (h + 1) * 4, :],
                        start=True,
                        stop=True,
                    )
                nc.scalar.copy(yts[s][:, so * OCT : (so + 1) * OCT, :], yb[:])
                if so == octs_per_sgroup - 1:
                    st_eng = st if s % 2 == 0 else nc.gpsimd
                    st_eng.dma_start(
                        y[:, s * sgroup : (s + 1) * sgroup, :], yts.pop(s)[:]
                    )

            for o in range(n_octs):
                g, og = divmod(o, octs_per_group)
                s = o // octs_per_sgroup
                if og == 0:
                    xt = xpool.tile([128, group, 128], F16, name="xt")
                    if g == 0:
                        # small first chunks so compute starts ASAP
                        nc.sync.dma_start(xt[:, :OCT, :], x[:, :OCT, :])
                        nc.sync.dma_start(xt[:, OCT : 2 * OCT, :], x[:, OCT : 2 * OCT, :])
                        nc.sync.dma_start(
                            xt[:, 2 * OCT :, :], x[:, 2 * OCT : group, :]
                        )
                    else:
                        nc.sync.dma_start(
                            xt[:], x[:, g * group : (g + 1) * group, :]
                        )
                    xts[g] = xt
                if o % octs_per_sgroup == 0:
                    yts[s] = ypool.tile([128, sgroup, 128], F16, name="yt")
                xt = xts[g]
                za = papool.tile([128, OCT, 128], F32)
                for j in range(OCT):
                    nc.tensor.matmul(
                        za[:, j, :],
                        lhsT=xt[:, og * OCT + j, :],
                        rhs=v_sb[:],
                        start=True,
                        stop=True,
                    )
                if o >= delay:
                    mm2_and_evacuate(o - delay)
                zsb = zpool.tile([128, OCT, 128], F16)
                nc.vector.tensor_copy(zsb[:], za[:])
                zsbs[o] = zsb
            for o in range(n_octs - delay, n_octs):
                mm2_and_evacuate(o)
    nc.compile()
    return nc


# ---------------------------------------------------------------------------
# fp32r baseline (previous best) kept for A/B comparison.

F32R = mybir.dt.float32r


def build_nc(n_tokens=T_CORE, mode="fp32rh", group=32, quad=4, lgran=0, sgran=0):
    r = mode in ("fp32r", "fp32rh")
    hostround = mode == "fp32rh"
    rwide = 256 if r else 128
    mmdt = F32R if r else F32
    dramdt = F32R if hostround else F32
    lgran = lgran or group
    sgran = sgran or group
    assert group % lgran == 0 and group % sgran == 0

    nc = bacc.Bacc("TRN2", target_bir_lowering=False, debug=False)
    x = nc.dram_tensor("x", [n_tokens, D], dramdt, kind="ExternalInput")
    u = nc.dram_tensor("u", [128, rwide], dramdt, kind="ExternalInput")
    v = nc.dram_tensor("v", [128, rwide], dramdt, kind="ExternalInput")
    y = nc.dram_tensor("y", [n_tokens, D], F32, kind="ExternalOutput")

    with tile.TileContext(nc) as tc:
        with (
            tc.tile_pool(name="const", bufs=1) as cpool,
            tc.tile_pool(name="xin", bufs=2) as xpool,
            tc.tile_pool(name="yout", bufs=2) as ypool,
            tc.tile_pool(name="pmid", bufs=4) as ppool,
            tc.tile_pool(name="ps", bufs=2, space="PSUM") as pspool,
        ):
            u_sb = cpool.tile([128, rwide], mmdt)
            v_sb = cpool.tile([128, rwide], mmdt)
            ld_const = (
                nc.gpsimd.dma_start if (r and not hostround) else nc.sync.dma_start
            )
            ld_const(u_sb[:], u[:])
            ld_const(v_sb[:], v[:])

            for g in range(n_tokens // group):
                xt = xpool.tile([128, group, 128], mmdt)
                ld_x = (
                    nc.gpsimd.dma_start if (r and not hostround) else nc.sync.dma_start
                )
                for c in range(group // lgran):
                    t0 = g * group + c * lgran
                    ld_x(
                        xt[:, c * lgran : (c + 1) * lgran, :],
                        x[t0 : t0 + lgran].rearrange("t (i1 i2) -> i1 t i2", i1=128),
                    )
                yt = ypool.tile([128, group, 128], F32)
                for q in range(group // quad):
                    pa = pspool.tile([128, quad, rwide], F32)
                    for j in range(quad):
                        nc.tensor.matmul(
                            pa[:, j, :],
                            lhsT=xt[:, q * quad + j, :],
                            rhs=u_sb[:],
                            start=True,
                            stop=True,
                        )
                    psb = ppool.tile([128, quad, 128], mmdt)
                    nc.vector.tensor_copy(psb[:], pa[:, :, 0:128])
                    pb = pspool.tile([128, quad, rwide], F32)
                    for j in range(quad):
                        nc.tensor.matmul(
                            pb[:, j, :],
                            lhsT=psb[:, j, :],
                            rhs=v_sb[:],
                            start=True,
                            stop=True,
                        )
                    nc.vector.tensor_copy(
                        yt[:, q * quad : (q + 1) * quad, :], pb[:, :, 0:128]
                    )
                for c in range(group // sgran):
                    t0 = g * group + c * sgran
                    nc.scalar.dma_start(
                        y[t0 : t0 + sgran].rearrange("t (j1 j2) -> j1 t j2", j1=128),
                        yt[:, c * sgran : (c + 1) * sgran, :],
                    )
    nc.compile()
    return nc


_NC_CACHE = {}


def _get_nc(n_tokens, mode, group, quad, lgran, sgran, store_eng):
    key = (n_tokens, mode, group, quad, lgran, sgran, store_eng)
    if key not in _NC_CACHE:
        if mode == "fp16t":
            _NC_CACHE[key] = build_nc_fp16t(n_tokens, group, quad, store_eng)
        else:
            _NC_CACHE[key] = build_nc(n_tokens, mode, group, quad, lgran, sgran)
    return _NC_CACHE[key]


def round_fp32r(a):
    u = np.ascontiguousarray(a, dtype=np.float32).view(np.uint32)
    r = ((u + np.uint32(0x800)) & np.uint32(0xFFFFF000)).view(np.float32)
    return np.where(np.isfinite(a), r, a).astype(np.float32)


def _prep_inputs(x, U, V, mode):
    x = np.ascontiguousarray(np.asarray(x), dtype=np.float32)
    U = np.ascontiguousarray(np.asarray(U), dtype=np.float32)
    V = np.ascontiguousarray(np.asarray(V), dtype=np.float32)
    if mode in ("fp32r", "fp32rh"):
        U = np.concatenate([U, U], axis=1)
        V = np.concatenate([V, V], axis=1)
    if mode == "fp32rh":
        x = round_fp32r(x)
        U = round_fp32r(U)
        V = round_fp32r(V)
    return x, U, V


def run(x, U, V, mode="fp16t", group=32, quad=4, lgran=0, sgran=0,
        store_eng="scalar", trace=False, **spmd_kwargs):
    """Shard over 8 cores, run, gather. Returns (y_full, BassKernelResults)."""
    if mode == "fp16t":
        x = np.ascontiguousarray(np.asarray(x), dtype=np.float32)
        T = x.shape[0]
        t_core = T // N_CORES
        nc = _get_nc(t_core, mode, group, quad, 0, 0, store_eng)
        # [t, i1*128+i2] -> [i2, t, i1] fp16
        xt = np.ascontiguousarray(
            x.reshape(T, 128, 128).transpose(2, 0, 1).astype(np.float16)
        )
        Uh = np.ascontiguousarray(np.asarray(U), dtype=np.float16)
        Vh = np.ascontiguousarray(np.asarray(V), dtype=np.float16)
        in_maps = [
            {
                "x": np.ascontiguousarray(xt[:, i * t_core : (i + 1) * t_core, :]),
                "u": Uh,
                "v": Vh,
            }
            for i in range(N_CORES)
        ]
        res = run_bass_kernel_spmd(
            nc, in_maps, list(range(N_CORES)), trace=trace, **spmd_kwargs
        )
        # y core result: [j1, t, j2] -> [t, j1, j2]
        out = np.concatenate(
            [
                res.results[i]["y"].transpose(1, 0, 2).reshape(t_core, D)
                for i in range(N_CORES)
            ],
            axis=0,
        ).astype(np.float32)
        return out, res

    x, U, V = _prep_inputs(x, U, V, mode)
    t_core = x.shape[0] // N_CORES
    nc = _get_nc(t_core, mode, group, quad, lgran, sgran, store_eng)
    in_maps = [
        {"x": x[i * t_core : (i + 1) * t_core], "u": U, "v": V}
        for i in range(N_CORES)
    ]
    res = run_bass_kernel_spmd(
        nc, in_maps, list(range(N_CORES)), trace=trace, **spmd_kwargs
    )
    out = np.concatenate([res.results[i]["y"] for i in range(N_CORES)], axis=0)
    return out, res


def kernel(x, U, V):
    out, _ = run(x, U, V)
    return out
